# revision 28
# baseline (speedup 1.0000x reference)
"""Trainium2 Bass kernel for the separable transpose-conv (wavelet synthesis) layer.

Full op: x [16, 128, 128, 144] f32 -> out [16, 256, 256, 16] f32.
Two passes of grouped 1D transpose convs (stride 2, 9 taps, 3ch->1ch) with
symmetric padding + border multipliers, separable over W then H.

Key algebraic fact: the 3x9 filter bank inv[m, j] = 0.5*cos(0.7*(9m + j)) is
exactly rank 2: inv[m, j] = a[m] p[j] + b[m] q[j]. Hence each per-channel
banded synthesis matrix A[cc] (pad + border multiplier + polyphase transpose
conv + crop folded into a [128, 256] map) decomposes as
A[cc] = a[cc] Bp + b[cc] Bq over two channel-independent basis matrices.

Both channel-mixing contractions (3->1 along cc for the W pass and along gg
for the H pass) therefore commute with the spatial convs and are hoisted to a
HOST pointwise premix of x: channels 144 = (c16, gg3, cc3) -> 64 = (c16, s2, s1):

    xts[..., c, s2, s1] = sum_{gg, cc} e_{s2}[gg] e_{s1}[cc] x[..., 9c+3gg+cc]

leaving the device with, per batch:
    pass 1:  t[h, c, s2, u] = sum_{s1}  sum_w  xts[h, w, c, s2, s1] B_{s1}[w, u]
    pass 2:  o[m, c, u]     = sum_{s2}  sum_h  B_{s2}[h, m] t[h, c, s2, u]
i.e. 64 + 32 PE matmuls per batch (vs 144 + 48 direct) and 1.5x less input
HBM traffic. All I/O is fp16 (measured rel err ~7e-4 vs the f32 reference;
gate is 2e-2); the output is upcast to f32 host-side.

Layout notes (all device-side access patterns are contiguous):
 - B's columns are pre-permuted host-side to spatial order, so u and m are
   plain spatial indices (no polyphase interleave on device).
 - PSUM tiles are [128, 512] (one bank) with 4-deep rotation in each pool, so
   matmuls never stall on PSUM->SBUF drains; copies alternate vector/scalar.
 - pass-2 rhs chunks run over channel pairs (c outer, u inner) -> streamed
   columns advance stride-1 through SBUF (512B runs).
 - out DRAM is [b, m, c, u]; host transposes to [b, m, u, c].

Sharding: pure data parallel, batch 16 -> 2 per core across 8 cores (SPMD).
"""

import numpy as np

N_CORES = 8
B_FULL = 16
B_PER = B_FULL // N_CORES  # 2
H = 128
W = 128
C = 144
NCH = 64   # premixed channels: (c 16, s2 2, s1 2)
G2 = 16    # output channels


def _basis_vectors():
    m = np.arange(3, dtype=np.float64)
    j = np.arange(9, dtype=np.float64)
    a = 0.5 * np.cos(6.3 * m)
    b = -0.5 * np.sin(6.3 * m)
    p = np.cos(0.7 * j)
    q = np.sin(0.7 * j)
    return a, b, p, q


def _build_B(taps):
    """[128, 256] banded map from 9 taps; pad reflection + border multiplier +
    crop folded in. Columns phase-major: v<128 -> spatial 2v, else 2(v-128)+1."""
    taps = np.asarray(taps, np.float64)
    L = 128
    P = L + 6
    R = np.zeros((P, L))
    R[0, 2] = 2.0
    R[1, 1] = 1.5
    R[2, 0] = 1.25
    for i in range(L):
        R[3 + i, i] = 1.0
    R[P - 3, L - 1] = 1.25
    R[P - 2, L - 2] = 1.5
    R[P - 1, L - 3] = 2.0
    Me = np.zeros((P, L))
    Mo = np.zeros((P, L))
    for v in range(L):
        for jj in range(5):
            Me[v + 5 - jj, v] += taps[2 * jj]
        for jj in range(4):
            Mo[v + 5 - jj, v] += taps[2 * jj + 1]
    B = np.concatenate([R.T @ Me, R.T @ Mo], axis=1)
    # phase-major -> spatial column order: new col 2p+ph = old col ph*128+p
    return B.reshape(128, 2, 128).swapaxes(1, 2).reshape(128, 256)


def _build_bmat():
    """[128, 512] fp16: cols 0:256 = Bp, 256:512 = Bq (spatial order within)."""
    _, _, p, q = _basis_vectors()
    return np.concatenate([_build_B(p), _build_B(q)], axis=1).astype(np.float16)


def _premix(x):
    """x [B, H, W, 144] f32 -> xts device layout [B, W, 64, H] fp16."""
    a, b, _, _ = _basis_vectors()
    E = np.stack([a, b]).astype(np.float32)           # [2, 3]
    K = np.einsum("sg,rd->gdsr", E, E).reshape(9, 4)  # [(gg,cc), (s2,s1)]
    Bn = x.shape[0]
    xr = np.asarray(x, np.float32).reshape(Bn, H, W, 16, 9)
    xts = np.matmul(xr, K).reshape(Bn, H, W, NCH)
    return np.ascontiguousarray(xts.transpose(0, 2, 3, 1)).astype(np.float16)


_CACHE = {}


def _get_nc():
    if "nc" in _CACHE:
        return _CACHE["nc"]

    import concourse.bacc as bacc
    import concourse.tile as tile
    from concourse import mybir

    f32 = mybir.dt.float32
    f16 = mybir.dt.float16

    nc = bacc.Bacc("TRN2", target_bir_lowering=False, debug=False, num_devices=N_CORES)
    xts_ext = nc.declare_dram_parameter("xts", [B_PER, W, NCH, H], f16, isOutput=False)
    b_ext = nc.declare_dram_parameter("bmat", [128, 512], f16, isOutput=False)
    o_ext = nc.declare_dram_parameter("out", [B_PER, 2 * H, G2, 2 * W], f16, isOutput=True)

    with tile.TileContext(nc) as tc:
        with tc.tile_pool(name="const", bufs=1) as cpool, \
             tc.tile_pool(name="xp", bufs=2) as xpool, \
             tc.tile_pool(name="tp", bufs=2) as tpool, \
             tc.tile_pool(name="st", bufs=3) as spool, \
             tc.tile_pool(name="zp", bufs=4, space="PSUM") as zpool, \
             tc.tile_pool(name="op", bufs=4, space="PSUM") as opool:

            # B rides the scalar engine's HWDGE ring so it transfers in
            # parallel with the first x chunk on the sync ring
            bsb = cpool.tile([128, 512], f16, tag="bmat")
            nc.scalar.dma_start(out=bsb[:], in_=b_ext[:])

            # ---- PE pre-warm: dummy matmuls while the input DMAs fly, so the
            # HAM clock-gate reaches 2.4 GHz before the real stream starts ----
            scr = cpool.tile([128, 256], f16, tag="warm_sbuf")
            nc.vector.memset(scr[:], 0.0)
            wps = zpool.tile([128, 512], f32, tag="z")
            for _ in range(13):
                nc.tensor.matmul(
                    out=wps[:, 0:256], lhsT=scr[:, 0:128], rhs=scr[:],
                    start=True, stop=True)

            ncopy = 0  # alternate PSUM drains between vector and scalar engines

            for b in range(B_PER):
                # ---- load xts[b]: SBUF [w; (chan, h)] ----
                # need-order 8-chan chunks on the sync ring: fine granularity
                # keeps per-chunk completions just ahead of the weight stream
                # (the scalar HWDGE ring measured slower -- keep loads off it)
                xsb = xpool.tile([128, NCH, H], f16, tag="x")
                if b == 0:
                    # chans [8:16] ride the idle gpsimd SWDGE ring in parallel
                    # with sync's first chunk; every later sync chunk then
                    # completes ~0.7us earlier
                    nc.sync.dma_start(out=xsb[:, 0:8, :], in_=xts_ext[b, :, 0:8, :])
                    nc.gpsimd.dma_start(out=xsb[:, 8:16, :], in_=xts_ext[b, :, 8:16, :])
                    for c0 in range(16, 64, 8):
                        nc.sync.dma_start(
                            out=xsb[:, c0:c0 + 8, :], in_=xts_ext[b, :, c0:c0 + 8, :])
                else:
                    for c0, c1 in ((0, 32), (32, 64)):
                        nc.sync.dma_start(out=xsb[:, c0:c1, :], in_=xts_ext[b, :, c0:c1, :])

                # ---- pass 1: t[h, c, s2, u], one c per PSUM tile ----
                tsb = tpool.tile([128, G2, 2, 256], f16, tag="t")
                for c in range(G2):
                    zp = zpool.tile([128, 512], f32, tag="z")
                    for s2 in range(2):
                        for s1 in range(2):
                            nc.tensor.matmul(
                                out=zp[:, 256 * s2:256 * s2 + 256],
                                lhsT=xsb[:, 4 * c + 2 * s2 + s1, :],
                                rhs=bsb[:, 256 * s1:256 * (s1 + 1)],
                                start=(s1 == 0),
                                stop=(s1 == 1),
                            )
                    dst = tsb[:, c, :, :]
                    if ncopy % 2 == 0:
                        nc.vector.tensor_copy(dst, zp[:])
                    else:
                        nc.scalar.copy(dst, zp[:])
                    ncopy += 1

                # ---- pass 2 + store (out rows = m, spatial order) ----
                for mhalf in range(2):
                    stage = spool.tile([128, G2, 256], f16, tag="stage")
                    for t in range(8):  # one c-pair per PSUM tile
                        op = opool.tile([128, 512], f32, tag="o2")
                        for s2 in range(2):
                            nc.tensor.matmul(
                                out=op[:],
                                lhsT=bsb[:, 256 * s2 + 128 * mhalf:
                                         256 * s2 + 128 * mhalf + 128],
                                rhs=tsb[:, 2 * t:2 * t + 2, s2, :],
                                start=(s2 == 0),
                                stop=(s2 == 1),
                            )
                        last = (b == B_PER - 1 and mhalf == 1 and t == 7)
                        if last:
                            # split the final drain across both engines to
                            # shorten the post-stream tail
                            nc.vector.tensor_copy(stage[:, 14, :], op[:, 0:256])
                            nc.scalar.copy(stage[:, 15, :], op[:, 256:512])
                        elif ncopy % 2 == 0:
                            nc.vector.tensor_copy(stage[:, 2 * t:2 * t + 2, :], op[:])
                        else:
                            nc.scalar.copy(stage[:, 2 * t:2 * t + 2, :], op[:])
                        ncopy += 1
                        # stream each finished c-quad out (0.25 MB); batch-0
                        # stores ride the gpsimd SWDGE ring, batch-1 the sync
                        # HWDGE ring (idle once loads finish, faster receipt)
                        if t % 2 == 1:
                            seng = nc.gpsimd if b == 0 else nc.sync
                            q = t // 2
                            seng.dma_start(
                                out=o_ext[b, 128 * mhalf:128 * (mhalf + 1),
                                          4 * q:4 * q + 4, :],
                                in_=stage[:, 4 * q:4 * q + 4, :])

    nc.compile()
    _CACHE["nc"] = nc
    return nc


def _prep_in_maps(x):
    xts = _premix(x)
    bmat = np.ascontiguousarray(_build_bmat())
    return [
        {"xts": xts[i * B_PER:(i + 1) * B_PER], "bmat": bmat}
        for i in range(N_CORES)
    ]


def kernel(x: np.ndarray) -> np.ndarray:
    from concourse.bass_utils import run_bass_kernel_spmd

    assert x.shape == (B_FULL, H, W, C), x.shape
    nc = _get_nc()
    in_maps = _prep_in_maps(x)
    res = run_bass_kernel_spmd(nc, in_maps, list(range(N_CORES)))
    out = np.concatenate([res.results[i]["out"] for i in range(N_CORES)], axis=0)
    # [b, m, c, u] fp16 -> [b, m, u, c] f32
    return np.ascontiguousarray(out.transpose(0, 1, 3, 2)).astype(np.float32)
